# revision 50
# baseline (speedup 1.0000x reference)
"""Trainium2 Bass kernel for the autoregressive GRU decode head.

Problem: context = mean over zones of encoded_features[:, -1]  -> (B, D)
then 12 autoregressive steps of a 2-layer GRU (H=256) + linear projection
to N=256 zones.  B=1024, data-parallel across 8 NeuronCores (128 batch each).

Structure (per core, feature-major / "transposed" activations):
  actT (128p, 4 slots, 128) bf16 : [h0 c0, h0 c1, h1 c0, h1 c1]
  Gate tensors (PSUM) use layout [p, c*128 + b].
  Matmuls: out(gate_chunk, B) = lhsT.T @ rhs, lhsT = W^T tile, rhs = actT
  slot, K-chunks accumulated in PSUM.  Pred feedback is folded into
  layer-0 weights (W_pred @ W_out).  ALL additive constants (step-emb
  matmul, gate biases, b_out) are injected as K=1 bias-row matmuls into
  the PSUM accumulation groups, so every sigmoid/tanh is a single wide
  ACT op and every elementwise op is a single wide DVE op.

Schedule: per step, the only matmuls that gate chain-0's sigmoid are the
8 "fold" MMs (h1-dependent); the h0-dependent half of the rz0 group is
emitted so it executes during the previous chain-1.  Off-critical-path
elementwise work (z*h, output-bias add) runs on GPSIMD.  Dependency-gated
dummy matmuls keep the PE HAM clock-gate warm (K=8/8) through the decode.

Phase 1 streams enc[:, -1] as bf16 in 8 x 32-zone chunks on both HWDGE
queues; the zone-sum tree alternates DVE / GPSIMD per chunk.
"""

import sys

for _p in ("/opt/trn_rl_repo",):
    if _p not in sys.path:
        sys.path.insert(0, _p)

import numpy as np
import ml_dtypes

import concourse.bass as bass
import concourse.tile as tile
from concourse import mybir
from concourse.vector_clock import ScopedClock

BF16 = ml_dtypes.bfloat16

B, T, NZ, D = 1024, 8, 256, 256
H = 256
STEPS = 12
N_CORES = 8
PC = B // N_CORES  # 128 batch per core

F32 = mybir.dt.float32
BF = mybir.dt.bfloat16
AF = mybir.ActivationFunctionType
OP = mybir.AluOpType

# ---- bias-row column map (each col is 128 wide) in brows [1, NBR*128] ----
C_RZ0 = 0                 # 12 steps x 4 cols
C_IN0 = C_RZ0 + 12 * 4    # 12 steps x 2
C_HN0 = C_IN0 + 12 * 2    # 2
C_RZ1 = C_HN0 + 2         # 4
C_IN1 = C_RZ1 + 4         # 2
C_HN1 = C_IN1 + 2         # 2
C_BOUT = C_HN1 + 2        # 2
NBR = C_BOUT + 2

# ---- packed-weight column offsets in w_all [128, WCOLS] (k-major) ----
O_RZ0 = 0                 # 4 k-chunks x 512
O_IN0 = O_RZ0 + 4 * 512   # 2 x 256
O_HN0 = O_IN0 + 2 * 256
O_RZ1 = O_HN0 + 2 * 256   # 4 x 512
O_IN1 = O_RZ1 + 4 * 512
O_HN1 = O_IN1 + 2 * 256
O_OUT = O_HN1 + 2 * 256   # 2 x 256
WCOLS = O_OUT + 2 * 256

# phase-1 chunk sizes (zones); even chunks ride the sync HWDGE queue
# (which also carries the weights first), odd chunks the scalar queue;
# byte-balanced so both queues finish together, tiny tail chunks so the
# final reduction after the last DMA byte is short
ZCHS = [16, 14, 16, 14, 16, 14, 16, 14, 16, 14, 16, 14, 16, 14,
        16, 8, 10, 6, 6]
NCH = len(ZCHS)


def _install_tile_drain_patch():
    """walrus (CoreV3) rejects >1 sync wait on the tail drain; spill extras
    onto preceding sync nops."""
    if getattr(tile.TileContext, "_drain_patch_installed", False):
        return

    def _patched(self, tick_clock, wait_clock):
        nc = self.nc
        bb = nc.cur_bb.bb
        drain_bi = nc.sync.drain()
        drain_inst = drain_bi.ins
        wait_clock.add_sem_waits(
            drain_inst, ScopedClock({None: tick_clock.global_clock})
        )
        w = drain_inst.sync_info.on_wait if drain_inst.sync_info else None
        maxw = 1
        if w and len(w) > maxw:
            extra = list(w[maxw:])
            drain_inst.sync_info.on_wait = list(w[:maxw])
            idx = bb.instructions.index(drain_inst)
            nops = []
            for i in range(0, len(extra), maxw):
                nop_bi = nc.sync.nop()
                nop = nop_bi.ins
                si = nop.sync_info
                nop.sync_info = mybir.SyncInfo(
                    on_wait=extra[i : i + maxw],
                    on_update=(si.on_update if si else []),
                )
                bb.instructions.remove(nop)
                nops.append(nop)
            bb.instructions[idx:idx] = nops
        nc.all_engine_barrier()
        popped = nc._tile_sem_poison_stack.pop()
        assert popped is self._sem_poison
        nc.clear_and_free_semaphores(list(self.sems.allocated().values()))
        nc.all_engine_barrier()

    tile.TileContext._drain_and_barrier = _patched
    tile.TileContext._drain_patch_installed = True


def _split_waits(nc, maxw=1):
    """This walrus build rejects instructions carrying more than ~1 sem
    wait; spill extra waits onto same-engine nops placed just before."""
    for bb in nc.main_func.blocks:
        new_list = []
        for inst in bb.instructions:
            si = inst.sync_info
            w = list(si.on_wait) if si and si.on_wait else []
            if len(w) > maxw:
                keep = w[len(w) - maxw:]
                extra = w[: len(w) - maxw]
                si.on_wait = keep
                for i in range(0, len(extra), maxw):
                    nop = mybir.InstNoOp(
                        name=f"{inst.name}-sw{i}", ins=[], outs=[]
                    )
                    nop.engine = inst.engine
                    nop.sync_info = mybir.SyncInfo(
                        on_wait=extra[i : i + maxw], on_update=[]
                    )
                    nc.register_instruction(nop)
                    new_list.append(nop)
            new_list.append(inst)
        bb.instructions[:] = new_list


class _Group:
    """start/stop flags for a PSUM accumulation group emitted in batches."""

    def __init__(self, total):
        self.total = total
        self.emitted = 0

    def flags(self):
        start = self.emitted == 0
        self.emitted += 1
        return start, self.emitted == self.total


def build_kernel(nsteps=STEPS, NDUM=1):
    _install_tile_drain_patch()
    nc = bass.Bass()

    enc = nc.declare_dram_parameter("enc", [PC, NZ, D], BF, isOutput=False)
    wall = nc.declare_dram_parameter("wall", [128, WCOLS], BF, isOutput=False)
    brows = nc.declare_dram_parameter("brows", [1, NBR * 128], BF, isOutput=False)
    out = nc.declare_dram_parameter("out", [PC, STEPS, NZ], F32, isOutput=True)

    with tile.TileContext(nc) as tc:
        with (
            tc.tile_pool(name="consts", bufs=1) as consts,
            tc.tile_pool(name="state", bufs=1) as state,
            tc.tile_pool(name="enc_pool", bufs=5) as enc_pool,
            tc.tile_pool(name="gates", bufs=2) as gates,
            tc.tile_pool(name="ostage", bufs=2) as ostage,
            tc.tile_pool(name="psum", bufs=1, space="PSUM") as psum,
        ):
            # ---------------- phase 1: DMAs ----------------
            # weights + bias rows first on the scalar HWDGE queue (fast,
            # done by ~20us); enc chunks split across both HWDGE queues
            brow_sb = consts.tile([1, NBR * 128], BF, tag="brow")
            nc.scalar.dma_start(brow_sb[:], brows[:])
            w_all = consts.tile([128, WCOLS], BF, tag="wall")
            nc.scalar.dma_start(w_all[:], wall[:])
            e_tiles = []
            z0 = 0
            for i, zch in enumerate(ZCHS):
                e_sb = enc_pool.tile([128, 16 * D], BF, tag="echunk", bufs=8)
                eng = nc.sync if i % 2 == 0 else nc.scalar
                eng.dma_start(e_sb[:, : zch * D], enc[:, z0 : z0 + zch, :])
                e_tiles.append(e_sb)
                z0 += zch
            ones_row = consts.tile([1, 128], BF, tag="ones")
            nc.gpsimd.memset(ones_row[:], 1.0)

            identity = consts.tile([128, 128], F32, tag="ident")
            nc.gpsimd.memset(identity[:], 0.0)
            nc.gpsimd.affine_select(
                out=identity[:],
                in_=identity[:],
                compare_op=OP.not_equal,
                fill=1.0,
                base=0,
                pattern=[[-1, 128]],
                channel_multiplier=1,
            )
            idbf = consts.tile([128, 128], BF, tag="idbf")
            nc.gpsimd.memset(idbf[:], 0.0)
            nc.gpsimd.affine_select(
                out=idbf[:],
                in_=idbf[:],
                compare_op=OP.not_equal,
                fill=1.0,
                base=0,
                pattern=[[-1, 128]],
                channel_multiplier=1,
            )
            # prewarm both ACT tables during phase 1
            warm = consts.tile([128, 2], F32, tag="warm")
            nc.scalar.activation(warm[:, 0:1], identity[:, 0:1], AF.Sigmoid)
            nc.scalar.activation(warm[:, 1:2], identity[:, 0:1], AF.Tanh)

            # PSUM junk bank + dummy-matmul helper (HAM warmers).  Each
            # dummy READS row 0 of a freshly produced tile (real dep, so
            # the scheduler cannot hoist it out of the window it fills)
            # with K=1 so it burns almost no power while keeping PE busy.
            junk = psum.tile([128, 512], F32, tag="junk", bufs=1)
            dmy_lhs = w_all[:, 0:128]
            # one never-resetting accumulation group for ALL dummies: a
            # fresh group per dummy costs ~270ns of PSUM turnaround; a
            # continuing group pipelines at the streaming floor (~107ns).
            # Values grow unboundedly but are never read.
            g_junk = _Group(2 * nsteps * 6 * max(NDUM, 1))

            def dum(n, rhs):
                w = rhs.free_size()
                for _ in range(n):
                    st, sp = g_junk.flags()
                    nc.tensor.matmul(junk[:, 0:w], dmy_lhs, rhs,
                                     start=st, stop=sp)

            # ---------------- phase 1: zone sums on the PE ----------------
            # sum over zone PAIRS = ONE accumulating identity-matmul group
            # (free=512, ~129 MMs) into the junk bank: pm[:,0:256] collects
            # even zones, pm[:,256:512] odd zones.  Exact f32 accumulation,
            # leaves DVE free, and cheap enough to hide under the DMA even
            # with the PE HAM clock-gate cold.
            pm_tp = psum.tile([128, 256], F32, tag="po", bufs=1)
            g_ctx = _Group(sum((zch + 1) // 2 for zch in ZCHS))
            for i, zch in enumerate(ZCHS):
                e_sb = e_tiles[i]
                for z in range(0, zch - 1, 2):
                    st, sp = g_ctx.flags()
                    nc.tensor.matmul(junk[:], idbf[:],
                                     e_sb[:, z * 256 : (z + 2) * 256],
                                     start=st, stop=sp)
                if zch % 2:
                    st, sp = g_ctx.flags()
                    nc.tensor.matmul(junk[:, 0:256], idbf[:],
                                     e_sb[:, (zch - 1) * 256 : zch * 256],
                                     start=st, stop=sp)
                if i < NCH - 3:
                    # filler matmuls (po bank) keep PE duty ~95% so the HAM
                    # clock-gate stays warm while tracking chunk arrivals
                    for _ in range(3):
                        nc.tensor.matmul(pm_tp[:], idbf[:], e_sb[:, 0:256],
                                         start=True, stop=True)
            ctx_t = state.tile([128, D], F32, tag="ctx")
            nc.vector.tensor_scalar_add(ctx_t[:], junk[:, 0:256], 0.0)
            nc.vector.tensor_tensor(ctx_t[:], ctx_t[:], junk[:, 256:512],
                                    OP.add)

            # ---------------- actT init: transpose + scale ----------------
            actT = state.tile([128, 4, 128], BF, tag="actT")
            for c in range(2):
                nc.tensor.transpose(
                    pm_tp[:, c * 128 : (c + 1) * 128],
                    ctx_t[:, c * 128 : (c + 1) * 128], identity[:]
                )
            actT_h0 = actT[:, 0:2, :].rearrange("p a b -> p (a b)")
            actT_h1 = actT[:, 2:4, :].rearrange("p a b -> p (a b)")
            nc.scalar.activation(actT_h0, pm_tp[:], AF.Copy, scale=1.0 / NZ)
            nc.scalar.activation(actT_h1, pm_tp[:], AF.Copy, scale=1.0 / NZ)
            actT_flat = actT[:, :, :].rearrange("p a b -> p (a b)")
            dum(4, actT_flat)

            # ---------------- decode helpers ----------------
            def bias_mms(pm, grp, colbase, nch):
                for c in range(nch):
                    st, sp = grp.flags()
                    nc.tensor.matmul(
                        pm[:, c * 128 : (c + 1) * 128],
                        brow_sb[0:1, (colbase + c) * 128 : (colbase + c + 1) * 128],
                        ones_row[0:1, :], start=st, stop=sp,
                    )

            def gate_mms(pm, grp, off, mdim, kis, slots, ms):
                for m in ms:
                    for ki, slot in zip(kis, slots):
                        st, sp = grp.flags()
                        nc.tensor.matmul(
                            pm[:, (m - ms[0]) * 128 : (m - ms[0] + 1) * 128],
                            w_all[:, off + ki * mdim + m * 128 :
                                  off + ki * mdim + (m + 1) * 128],
                            actT[:, slot, :], start=st, stop=sp,
                        )

            def chain(layer, pm_r, pm_z, hn_bf, pm_in, after_t=None,
                      mid=None, dup=None):
                s_ = gates.tile([128, 512], BF, tag=f"s{layer}")
                nc.scalar.activation(s_[:, 0:256], pm_r[:], AF.Sigmoid)
                dum(NDUM, s_[:, 0:256])
                nc.scalar.activation(s_[:, 256:512], pm_z[:], AF.Sigmoid)
                dum(NDUM, s_[:, :])
                hv = actT[:, 2 * layer : 2 * layer + 2, :].rearrange(
                    "p a b -> p (a b)")
                # c_ = z * h_prev on GPSIMD (needed only at the h-update)
                c_ = gates.tile([128, 256], BF, tag=f"c{layer}")
                nc.gpsimd.tensor_tensor(c_[:], s_[:, 256:512], hv, OP.mult)
                t_ = gates.tile([128, 256], BF, tag=f"t{layer}")
                nc.vector.tensor_tensor(t_[:], hn_bf[:], s_[:, 0:256], OP.mult)
                dum(NDUM, t_[:])
                if after_t is not None:
                    after_t()
                v_ = gates.tile([128, 256], BF, tag=f"v{layer}")
                n_ = gates.tile([128, 256], BF, tag=f"n{layer}")
                zm = gates.tile([128, 256], BF, tag=f"zm{layer}")
                # v/tanh/zm/h run as two 128-col halves pipelined across
                # DVE and ACT so the h-update lands earlier
                h0c = slice(0, 128)
                h1c = slice(128, 256)
                nc.vector.tensor_tensor(v_[:, h0c], pm_in[:, h0c], t_[:, h0c],
                                        OP.add)
                nc.vector.tensor_tensor(v_[:, h1c], pm_in[:, h1c], t_[:, h1c],
                                        OP.add)
                dum(NDUM, v_[:])
                if mid is not None:
                    mid()  # other layer's hn -> bf16 copy (DVE dead time)
                nc.scalar.activation(n_[:, h0c], v_[:, h0c], AF.Tanh)
                nc.scalar.activation(n_[:, h1c], v_[:, h1c], AF.Tanh)
                dum(NDUM, n_[:])
                for cs in (h0c, h1c):
                    nc.vector.scalar_tensor_tensor(
                        zm[:, cs], s_[:, 256 + cs.start : 256 + cs.stop], 1.0,
                        n_[:, cs], op0=OP.subtract, op1=OP.mult,
                    )
                dum(NDUM, zm[:])
                for cs in (h0c, h1c):
                    nc.vector.tensor_tensor(hv[:, cs], c_[:, cs], zm[:, cs],
                                            OP.subtract)
                if dup is not None:
                    # independent copy of the new h for consumers that must
                    # not WAR-block the next h-update (wout reads this)
                    nc.vector.tensor_tensor(dup, c_[:], zm[:], OP.subtract)

            # wout reads h1 from its own copy (h1cp) so the next chain1
            # h-update never WAR-waits on it
            h1cp = state.tile([128, 2, 128], BF, tag="h1cp")
            h1cp_flat = h1cp[:, :, :].rearrange("p a b -> p (a b)")

            def wout_mms():
                pm_out = psum.tile([128, 256], F32, tag="po", bufs=1)
                g = _Group(1 + 2)
                st, sp = g.flags()
                nc.tensor.matmul(
                    pm_out[:], ones_row[0:1, :],
                    brow_sb[0:1, C_BOUT * 128 : (C_BOUT + 2) * 128],
                    start=st, stop=sp,
                )
                for c in range(2):
                    st, sp = g.flags()
                    nc.tensor.matmul(
                        pm_out[:], h1cp[:, c, :],
                        w_all[:, O_OUT + c * 256 : O_OUT + (c + 1) * 256],
                        start=st, stop=sp,
                    )
                return pm_out

            # virtual-time pins: generous over-estimates so the scheduler
            # slots the (off-critical-path) output stores AFTER the chain
            # ops of their step, never between them
            PH1_EST_US, STEP_EST_US = 60.0, 6.8

            def wout_store(ti, pm_out):
                ms = (PH1_EST_US + (ti + 1.8) * STEP_EST_US) / 1000.0
                with tc.tile_wait_until(ms):
                    o_ = ostage.tile([128, 256], F32, tag="o")
                    nc.vector.tensor_scalar_add(o_[:], pm_out[:], 0.0)
                    nc.sync.dma_start(out[:, ti, :], o_[:])

            # ---------------- decode loop ----------------
            # PSUM tiles: r0,z0,r1,z1 own banks; hn0/hn1 share one bank
            # (tag hnx), in0/in1 share one (tag inx) — their lifetimes
            # alternate within a step.
            prev = None  # (pm_r1, pm_z1, pm_hn1, pm_in1) of step t-1
            for t in range(nsteps):
                # -- P(t): h0'(t-1)-dependent MMs (run during chain1(t-1)) --
                pm_r0 = psum.tile([128, 256], F32, tag="r0", bufs=1)
                pm_z0 = psum.tile([128, 256], F32, tag="z0", bufs=1)
                g_r0 = _Group(2 + 4 + (4 if t else 0))
                g_z0 = _Group(2 + 4 + (4 if t else 0))
                bias_mms(pm_r0, g_r0, C_RZ0 + 4 * t, 2)
                gate_mms(pm_r0, g_r0, O_RZ0, 512, (2, 3), (0, 1), (0, 1))
                bias_mms(pm_z0, g_z0, C_RZ0 + 4 * t + 2, 2)
                gate_mms(pm_z0, g_z0, O_RZ0, 512, (2, 3), (0, 1), (2, 3))

                # hn0(t) is emitted via chain1(t-1)'s after_t hook: it
                # reuses the hnx bank whose previous value (hn1(t-1))
                # chain1's t_ op reads.  The bf16 copy rides chain1's
                # mid-slot (DVE dead time under tanh).
                pm_hn0 = psum.tile([128, 256], F32, tag="hnx", bufs=1)
                g_hn0 = _Group(2 + 4)
                hn0_bf = gates.tile([128, 256], BF, tag="hnb0")

                def emit_hn0():
                    bias_mms(pm_hn0, g_hn0, C_HN0, 2)
                    gate_mms(pm_hn0, g_hn0, O_HN0, 256, (0, 1), (0, 1), (0, 1))

                def copy_hn0():
                    nc.vector.tensor_scalar_add(hn0_bf[:], pm_hn0[:], 0.0)

                # -- chain1(t-1) --
                if t:
                    chain(1, prev[0], prev[1], prev[2], prev[3],
                          after_t=emit_hn0, mid=copy_hn0, dup=h1cp_flat)
                else:
                    emit_hn0()
                    copy_hn0()

                # -- F(t): h1'(t-1)-dependent MMs --
                pm_in0 = psum.tile([128, 256], F32, tag="inx", bufs=1)
                g_in0 = _Group(2 + (4 if t else 0))
                if t:
                    # fold parts close the r0/z0 groups -> release sigmoids
                    gate_mms(pm_r0, g_r0, O_RZ0, 512, (0, 1), (2, 3), (0, 1))
                    gate_mms(pm_z0, g_z0, O_RZ0, 512, (0, 1), (2, 3), (2, 3))
                bias_mms(pm_in0, g_in0, C_IN0 + 2 * t, 2)
                pm_po = None
                if t:
                    gate_mms(pm_in0, g_in0, O_IN0, 256, (0, 1), (2, 3), (0, 1))
                    pm_po = wout_mms()
                pm_r1 = psum.tile([128, 256], F32, tag="r1", bufs=1)
                pm_z1 = psum.tile([128, 256], F32, tag="z1", bufs=1)
                g_r1 = _Group(2 + 4 + 4)
                g_z1 = _Group(2 + 4 + 4)
                bias_mms(pm_r1, g_r1, C_RZ1, 2)
                gate_mms(pm_r1, g_r1, O_RZ1, 512, (2, 3), (2, 3), (0, 1))
                bias_mms(pm_z1, g_z1, C_RZ1 + 2, 2)
                gate_mms(pm_z1, g_z1, O_RZ1, 512, (2, 3), (2, 3), (2, 3))

                # hn1(t) via chain0(t)'s after_t hook (hnx-bank rotation)
                pm_hn1 = psum.tile([128, 256], F32, tag="hnx", bufs=1)
                g_hn1 = _Group(2 + 4)
                hn1_bf = gates.tile([128, 256], BF, tag="hnb1")

                def emit_hn1():
                    bias_mms(pm_hn1, g_hn1, C_HN1, 2)
                    gate_mms(pm_hn1, g_hn1, O_HN1, 256, (0, 1), (2, 3), (0, 1))

                def copy_hn1():
                    nc.vector.tensor_scalar_add(hn1_bf[:], pm_hn1[:], 0.0)

                # -- chain0(t) --
                chain(0, pm_r0, pm_z0, hn0_bf, pm_in0, after_t=emit_hn1,
                      mid=copy_hn1)
                if pm_po is not None:
                    wout_store(t - 1, pm_po)

                # -- I(t): h0'(t)-dependent MMs --
                pm_in1 = psum.tile([128, 256], F32, tag="inx", bufs=1)
                g_in1 = _Group(2 + 4)
                gate_mms(pm_r1, g_r1, O_RZ1, 512, (0, 1), (0, 1), (0, 1))
                gate_mms(pm_z1, g_z1, O_RZ1, 512, (0, 1), (0, 1), (2, 3))
                bias_mms(pm_in1, g_in1, C_IN1, 2)
                gate_mms(pm_in1, g_in1, O_IN1, 256, (0, 1), (0, 1), (0, 1))
                prev = (pm_r1, pm_z1, hn1_bf, pm_in1)

            # -- tail: chain1(last) + its output --
            chain(1, prev[0], prev[1], prev[2], prev[3], dup=h1cp_flat)
            pm_po = wout_mms()
            wout_store(nsteps - 1, pm_po)

    _split_waits(nc)
    return nc


def _prep_inputs(encoded_features, step_emb, W_ih0, W_hh0, b_ih0, b_hh0,
                 W_ih1, W_hh1, b_ih1, b_hh1, W_out, b_out):
    """Host-side staging: slice/shard the big input, transpose + cast
    weights, fold the output projection into layer-0 input weights, fold
    the step-embedding matmul + all additive constants into bias rows."""
    f4 = np.float32
    enc_last = np.asarray(encoded_features)[:, -1].astype(BF16)
    enc_last = np.ascontiguousarray(enc_last)

    W_ih0 = np.asarray(W_ih0, f4)
    W_hh0 = np.asarray(W_hh0, f4)
    W_ih1 = np.asarray(W_ih1, f4)
    W_hh1 = np.asarray(W_hh1, f4)
    W_out = np.asarray(W_out, f4)
    step_emb = np.asarray(step_emb, f4)
    b_ih0 = np.asarray(b_ih0, f4)
    b_hh0 = np.asarray(b_hh0, f4)
    b_ih1 = np.asarray(b_ih1, f4)
    b_hh1 = np.asarray(b_hh1, f4)
    b_out = np.asarray(b_out, f4)

    W_emb = W_ih0[:, :D]          # (768, 256)
    W_pred = W_ih0[:, D:]         # (768, 256)
    W_fold = W_pred @ W_out       # (768, 256)
    b_fold = W_pred @ b_out       # (768,)

    gi_emb = step_emb[:STEPS] @ W_emb.T + b_ih0[None, :]   # (12, 768)

    def kstack(*mats_cols):
        chunks = []
        for mat, cols in mats_cols:
            mt = np.ascontiguousarray(mat.T[:, cols])  # (K, M)
            for k in range(0, mt.shape[0], 128):
                chunks.append(mt[k : k + 128])
        return np.stack(chunks).astype(BF16)  # (nk, 128, M)

    rz = slice(0, 512)
    ng = slice(512, 768)
    wrz0 = kstack((W_fold, rz), (W_hh0, rz))   # K: h1c0,h1c1,h0c0,h0c1
    win0 = kstack((W_fold, ng))
    whn0 = kstack((W_hh0, ng))
    wrz1 = kstack((W_ih1, rz), (W_hh1, rz))    # K: h0c0,h0c1,h1c0,h1c1
    win1 = kstack((W_ih1, ng))
    whn1 = kstack((W_hh1, ng))
    wout = np.stack([np.ascontiguousarray(W_out.T)[k : k + 128]
                     for k in (0, 128)]).astype(BF16)      # (2, 128, 256)

    w_pack = np.concatenate(
        [w.transpose(1, 0, 2).reshape(128, -1)
         for w in (wrz0, win0, whn0, wrz1, win1, whn1, wout)],
        axis=1,
    )
    assert w_pack.shape == (128, WCOLS)

    brows = np.zeros(NBR * 128, f4)

    def put(base, vec):
        brows[base * 128 : base * 128 + len(vec)] = vec

    for t in range(STEPS):
        extra = b_fold if t > 0 else np.zeros_like(b_fold)
        put(C_RZ0 + t * 4, gi_emb[t, :512] + b_hh0[:512] + extra[:512])
        put(C_IN0 + t * 2, gi_emb[t, 512:] + extra[512:])
    put(C_HN0, b_hh0[512:])
    put(C_RZ1, b_ih1[:512] + b_hh1[:512])
    put(C_IN1, b_ih1[512:])
    put(C_HN1, b_hh1[512:])
    put(C_BOUT, b_out)
    brows = brows.astype(BF16)[None, :]

    shared = dict(wall=np.ascontiguousarray(w_pack), brows=brows)
    in_maps = []
    for i in range(N_CORES):
        m = dict(shared)
        m["enc"] = enc_last[i * PC : (i + 1) * PC]
        in_maps.append(m)
    return in_maps


_CACHE = {}


def _run(in_maps, trace=False):
    from concourse.bass_utils import run_bass_kernel_spmd

    if "nc" not in _CACHE:
        _CACHE["nc"] = build_kernel()
    nc = _CACHE["nc"]
    res = run_bass_kernel_spmd(
        nc, in_maps, core_ids=list(range(N_CORES)), trace=trace
    )
    preds = np.concatenate([res.results[i]["out"] for i in range(N_CORES)], axis=0)
    return preds, res


def kernel(encoded_features, step_emb, W_ih0, W_hh0, b_ih0, b_hh0,
           W_ih1, W_hh1, b_ih1, b_hh1, W_out, b_out, num_steps):
    assert int(num_steps) == STEPS
    in_maps = _prep_inputs(encoded_features, step_emb, W_ih0, W_hh0, b_ih0,
                           b_hh0, W_ih1, W_hh1, b_ih1, b_hh1, W_out, b_out)
    preds, _ = _run(in_maps, trace=False)
    return preds


# revision 51
# speedup vs baseline: 1.1978x; 1.1978x over previous
"""Trainium2 Bass kernel for the autoregressive GRU decode head.

Problem: context = mean over zones of encoded_features[:, -1]  -> (B, D)
then 12 autoregressive steps of a 2-layer GRU (H=256) + linear projection
to N=256 zones.  B=1024, data-parallel across 8 NeuronCores (128 batch each).

Structure (per core, feature-major / "transposed" activations):
  actT (128p, 4 slots, 128) bf16 : [h0 c0, h0 c1, h1 c0, h1 c1]
  Gate tensors (PSUM) use layout [p, c*128 + b].
  Matmuls: out(gate_chunk, B) = lhsT.T @ rhs, lhsT = W^T tile, rhs = actT
  slot, K-chunks accumulated in PSUM.  Pred feedback is folded into
  layer-0 weights (W_pred @ W_out).  ALL additive constants (step-emb
  matmul, gate biases, b_out) are injected as K=1 bias-row matmuls into
  the PSUM accumulation groups, so every sigmoid/tanh is a single wide
  ACT op and every elementwise op is a single wide DVE op.

Schedule: per step, the only matmuls that gate chain-0's sigmoid are the
8 "fold" MMs (h1-dependent); the h0-dependent half of the rz0 group is
emitted so it executes during the previous chain-1.  Off-critical-path
elementwise work (z*h, output-bias add) runs on GPSIMD.  Dependency-gated
dummy matmuls keep the PE HAM clock-gate warm (K=8/8) through the decode.

Phase 1 streams enc[:, -1] as bf16 in 8 x 32-zone chunks on both HWDGE
queues; the zone-sum tree alternates DVE / GPSIMD per chunk.
"""

import sys

for _p in ("/opt/trn_rl_repo",):
    if _p not in sys.path:
        sys.path.insert(0, _p)

import numpy as np
import ml_dtypes

import concourse.bass as bass
import concourse.tile as tile
from concourse import mybir
from concourse.vector_clock import ScopedClock

BF16 = ml_dtypes.bfloat16

B, T, NZ, D = 1024, 8, 256, 256
H = 256
STEPS = 12
N_CORES = 8
PC = B // N_CORES  # 128 batch per core

F32 = mybir.dt.float32
BF = mybir.dt.bfloat16
AF = mybir.ActivationFunctionType
OP = mybir.AluOpType

# ---- bias-row column map (each col is 128 wide) in brows [1, NBR*128] ----
C_RZ0 = 0                 # 12 steps x 4 cols
C_IN0 = C_RZ0 + 12 * 4    # 12 steps x 2
C_HN0 = C_IN0 + 12 * 2    # 2
C_RZ1 = C_HN0 + 2         # 4
C_IN1 = C_RZ1 + 4         # 2
C_HN1 = C_IN1 + 2         # 2
C_BOUT = C_HN1 + 2        # 2
NBR = C_BOUT + 2

# ---- packed-weight column offsets in w_all [128, WCOLS] (k-major) ----
O_RZ0 = 0                 # 4 k-chunks x 512
O_IN0 = O_RZ0 + 4 * 512   # 2 x 256
O_HN0 = O_IN0 + 2 * 256
O_RZ1 = O_HN0 + 2 * 256   # 4 x 512
O_IN1 = O_RZ1 + 4 * 512
O_HN1 = O_IN1 + 2 * 256
O_OUT = O_HN1 + 2 * 256   # 2 x 256
WCOLS = O_OUT + 2 * 256

# phase-1 chunk sizes (zones); even chunks ride the sync HWDGE queue
# (which also carries the weights first), odd chunks the scalar queue;
# byte-balanced so both queues finish together, tiny tail chunks so the
# final reduction after the last DMA byte is short
ZCHS = [16, 14, 16, 14, 16, 14, 16, 14, 16, 14, 16, 14, 16, 14,
        16, 8, 10, 6, 6]
NCH = len(ZCHS)


def _install_tile_drain_patch():
    """walrus (CoreV3) rejects >1 sync wait on the tail drain; spill extras
    onto preceding sync nops."""
    if getattr(tile.TileContext, "_drain_patch_installed", False):
        return

    def _patched(self, tick_clock, wait_clock):
        nc = self.nc
        bb = nc.cur_bb.bb
        drain_bi = nc.sync.drain()
        drain_inst = drain_bi.ins
        wait_clock.add_sem_waits(
            drain_inst, ScopedClock({None: tick_clock.global_clock})
        )
        w = drain_inst.sync_info.on_wait if drain_inst.sync_info else None
        maxw = 1
        if w and len(w) > maxw:
            extra = list(w[maxw:])
            drain_inst.sync_info.on_wait = list(w[:maxw])
            idx = bb.instructions.index(drain_inst)
            nops = []
            for i in range(0, len(extra), maxw):
                nop_bi = nc.sync.nop()
                nop = nop_bi.ins
                si = nop.sync_info
                nop.sync_info = mybir.SyncInfo(
                    on_wait=extra[i : i + maxw],
                    on_update=(si.on_update if si else []),
                )
                bb.instructions.remove(nop)
                nops.append(nop)
            bb.instructions[idx:idx] = nops
        nc.all_engine_barrier()
        popped = nc._tile_sem_poison_stack.pop()
        assert popped is self._sem_poison
        nc.clear_and_free_semaphores(list(self.sems.allocated().values()))
        nc.all_engine_barrier()

    tile.TileContext._drain_and_barrier = _patched
    tile.TileContext._drain_patch_installed = True


def _split_waits(nc, maxw=1):
    """This walrus build rejects instructions carrying more than ~1 sem
    wait; spill extra waits onto same-engine nops placed just before."""
    for bb in nc.main_func.blocks:
        new_list = []
        for inst in bb.instructions:
            si = inst.sync_info
            w = list(si.on_wait) if si and si.on_wait else []
            if len(w) > maxw:
                keep = w[len(w) - maxw:]
                extra = w[: len(w) - maxw]
                si.on_wait = keep
                for i in range(0, len(extra), maxw):
                    nop = mybir.InstNoOp(
                        name=f"{inst.name}-sw{i}", ins=[], outs=[]
                    )
                    nop.engine = inst.engine
                    nop.sync_info = mybir.SyncInfo(
                        on_wait=extra[i : i + maxw], on_update=[]
                    )
                    nc.register_instruction(nop)
                    new_list.append(nop)
            new_list.append(inst)
        bb.instructions[:] = new_list


class _Group:
    """start/stop flags for a PSUM accumulation group emitted in batches."""

    def __init__(self, total):
        self.total = total
        self.emitted = 0

    def flags(self):
        start = self.emitted == 0
        self.emitted += 1
        return start, self.emitted == self.total


def build_kernel(nsteps=STEPS, NDUM=1):
    _install_tile_drain_patch()
    nc = bass.Bass()

    enc = nc.declare_dram_parameter("enc", [PC, NZ, D], BF, isOutput=False)
    wall = nc.declare_dram_parameter("wall", [128, WCOLS], BF, isOutput=False)
    brows = nc.declare_dram_parameter("brows", [1, NBR * 128], BF, isOutput=False)
    out = nc.declare_dram_parameter("out", [PC, STEPS, NZ], F32, isOutput=True)

    with tile.TileContext(nc) as tc:
        with (
            tc.tile_pool(name="consts", bufs=1) as consts,
            tc.tile_pool(name="state", bufs=1) as state,
            tc.tile_pool(name="enc_pool", bufs=5) as enc_pool,
            tc.tile_pool(name="gates", bufs=2) as gates,
            tc.tile_pool(name="ostage", bufs=2) as ostage,
            tc.tile_pool(name="psum", bufs=1, space="PSUM") as psum,
        ):
            # ---------------- phase 1: DMAs ----------------
            # weights + bias rows first on the scalar HWDGE queue (fast,
            # done by ~20us); enc chunks split across both HWDGE queues
            brow_sb = consts.tile([1, NBR * 128], BF, tag="brow")
            nc.scalar.dma_start(brow_sb[:], brows[:])
            w_all = consts.tile([128, WCOLS], BF, tag="wall")
            nc.scalar.dma_start(w_all[:], wall[:])
            e_tiles = []
            z0 = 0
            for i, zch in enumerate(ZCHS):
                e_sb = enc_pool.tile([128, 16 * D], BF, tag="echunk", bufs=8)
                eng = nc.sync if i % 2 == 0 else nc.scalar
                eng.dma_start(e_sb[:, : zch * D], enc[:, z0 : z0 + zch, :])
                e_tiles.append(e_sb)
                z0 += zch
            ones_row = consts.tile([1, 128], BF, tag="ones")
            nc.gpsimd.memset(ones_row[:], 1.0)

            identity = consts.tile([128, 128], F32, tag="ident")
            nc.gpsimd.memset(identity[:], 0.0)
            nc.gpsimd.affine_select(
                out=identity[:],
                in_=identity[:],
                compare_op=OP.not_equal,
                fill=1.0,
                base=0,
                pattern=[[-1, 128]],
                channel_multiplier=1,
            )
            idbf = consts.tile([128, 128], BF, tag="idbf")
            nc.gpsimd.memset(idbf[:], 0.0)
            nc.gpsimd.affine_select(
                out=idbf[:],
                in_=idbf[:],
                compare_op=OP.not_equal,
                fill=1.0,
                base=0,
                pattern=[[-1, 128]],
                channel_multiplier=1,
            )
            # prewarm both ACT tables during phase 1
            warm = consts.tile([128, 2], F32, tag="warm")
            nc.scalar.activation(warm[:, 0:1], identity[:, 0:1], AF.Sigmoid)
            nc.scalar.activation(warm[:, 1:2], identity[:, 0:1], AF.Tanh)

            # PSUM junk bank + dummy-matmul helper (HAM warmers).  Each
            # dummy READS row 0 of a freshly produced tile (real dep, so
            # the scheduler cannot hoist it out of the window it fills)
            # with K=1 so it burns almost no power while keeping PE busy.
            junk = psum.tile([128, 512], F32, tag="junk", bufs=1)
            dmy_lhs = w_all[:, 0:128]
            # one never-resetting accumulation group for ALL dummies: a
            # fresh group per dummy costs ~270ns of PSUM turnaround; a
            # continuing group pipelines at the streaming floor (~107ns).
            # Values grow unboundedly but are never read.
            g_junk = _Group(2 * nsteps * 6 * max(NDUM, 1))

            def dum(n, rhs):
                w = rhs.free_size()
                for _ in range(n):
                    st, sp = g_junk.flags()
                    nc.tensor.matmul(junk[:, 0:w], dmy_lhs, rhs,
                                     start=st, stop=sp)

            # ---------------- phase 1: zone sums on the PE ----------------
            # sum over zone PAIRS = ONE accumulating identity-matmul group
            # (free=512, ~129 MMs) into the junk bank: pm[:,0:256] collects
            # even zones, pm[:,256:512] odd zones.  Exact f32 accumulation,
            # leaves DVE free, and cheap enough to hide under the DMA even
            # with the PE HAM clock-gate cold.
            pm_tp = psum.tile([128, 256], F32, tag="po", bufs=1)
            g_ctx = _Group(sum((zch + 1) // 2 for zch in ZCHS))
            for i, zch in enumerate(ZCHS):
                e_sb = e_tiles[i]
                for z in range(0, zch - 1, 2):
                    st, sp = g_ctx.flags()
                    nc.tensor.matmul(junk[:], idbf[:],
                                     e_sb[:, z * 256 : (z + 2) * 256],
                                     start=st, stop=sp)
                if zch % 2:
                    st, sp = g_ctx.flags()
                    nc.tensor.matmul(junk[:, 0:256], idbf[:],
                                     e_sb[:, (zch - 1) * 256 : zch * 256],
                                     start=st, stop=sp)
                if i < NCH - 3:
                    # filler matmuls (po bank) keep PE duty ~95% so the HAM
                    # clock-gate stays warm while tracking chunk arrivals
                    for _ in range(3):
                        nc.tensor.matmul(pm_tp[:], idbf[:], e_sb[:, 0:256],
                                         start=True, stop=True)
            ctx_t = state.tile([128, D], F32, tag="ctx")
            nc.vector.tensor_scalar_add(ctx_t[:], junk[:, 0:256], 0.0)
            nc.vector.tensor_tensor(ctx_t[:], ctx_t[:], junk[:, 256:512],
                                    OP.add)

            # ---------------- actT init: transpose + scale ----------------
            actT = state.tile([128, 4, 128], BF, tag="actT")
            for c in range(2):
                nc.tensor.transpose(
                    pm_tp[:, c * 128 : (c + 1) * 128],
                    ctx_t[:, c * 128 : (c + 1) * 128], identity[:]
                )
            actT_h0 = actT[:, 0:2, :].rearrange("p a b -> p (a b)")
            actT_h1 = actT[:, 2:4, :].rearrange("p a b -> p (a b)")
            nc.scalar.activation(actT_h0, pm_tp[:], AF.Copy, scale=1.0 / NZ)
            nc.scalar.activation(actT_h1, pm_tp[:], AF.Copy, scale=1.0 / NZ)
            actT_flat = actT[:, :, :].rearrange("p a b -> p (a b)")
            dum(4, actT_flat)

            # ---------------- decode helpers ----------------
            def bias_mms(pm, grp, colbase, nch):
                for c in range(nch):
                    st, sp = grp.flags()
                    nc.tensor.matmul(
                        pm[:, c * 128 : (c + 1) * 128],
                        brow_sb[0:1, (colbase + c) * 128 : (colbase + c + 1) * 128],
                        ones_row[0:1, :], start=st, stop=sp,
                    )

            def gate_mms(pm, grp, off, mdim, kis, slots, ms):
                for m in ms:
                    for ki, slot in zip(kis, slots):
                        st, sp = grp.flags()
                        nc.tensor.matmul(
                            pm[:, (m - ms[0]) * 128 : (m - ms[0] + 1) * 128],
                            w_all[:, off + ki * mdim + m * 128 :
                                  off + ki * mdim + (m + 1) * 128],
                            actT[:, slot, :], start=st, stop=sp,
                        )

            def chain(layer, pm_r, pm_z, hn_bf, pm_in, after_t=None,
                      mid=None, dup=None):
                s_ = gates.tile([128, 512], BF, tag=f"s{layer}")
                nc.scalar.activation(s_[:, 0:256], pm_r[:], AF.Sigmoid)
                dum(NDUM, s_[:, 0:256])
                nc.scalar.activation(s_[:, 256:512], pm_z[:], AF.Sigmoid)
                dum(NDUM, s_[:, :])
                hv = actT[:, 2 * layer : 2 * layer + 2, :].rearrange(
                    "p a b -> p (a b)")
                # c_ = z * h_prev on GPSIMD (needed only at the h-update)
                c_ = gates.tile([128, 256], BF, tag=f"c{layer}")
                nc.gpsimd.tensor_tensor(c_[:], s_[:, 256:512], hv, OP.mult)
                t_ = gates.tile([128, 256], BF, tag=f"t{layer}")
                nc.vector.tensor_tensor(t_[:], hn_bf[:], s_[:, 0:256], OP.mult)
                dum(NDUM, t_[:])
                if after_t is not None:
                    after_t()
                v_ = gates.tile([128, 256], BF, tag=f"v{layer}")
                nc.vector.tensor_tensor(v_[:], pm_in[:], t_[:], OP.add)
                dum(NDUM, v_[:])
                if mid is not None:
                    mid()  # other layer's hn -> bf16 copy (DVE dead time)
                n_ = gates.tile([128, 256], BF, tag=f"n{layer}")
                nc.scalar.activation(n_[:], v_[:], AF.Tanh)
                dum(NDUM, n_[:])
                zm = gates.tile([128, 256], BF, tag=f"zm{layer}")
                nc.vector.scalar_tensor_tensor(
                    zm[:], s_[:, 256:512], 1.0, n_[:],
                    op0=OP.subtract, op1=OP.mult,
                )
                dum(NDUM, zm[:])
                nc.vector.tensor_tensor(hv, c_[:], zm[:], OP.subtract)
                if dup is not None:
                    # independent copy of the new h for consumers that must
                    # not WAR-block the next h-update (wout reads this)
                    nc.vector.tensor_tensor(dup, c_[:], zm[:], OP.subtract)

            # wout reads h1 from its own copy (h1cp) so the next chain1
            # h-update never WAR-waits on it
            h1cp = state.tile([128, 2, 128], BF, tag="h1cp")
            h1cp_flat = h1cp[:, :, :].rearrange("p a b -> p (a b)")

            def wout_mms():
                pm_out = psum.tile([128, 256], F32, tag="po", bufs=1)
                g = _Group(1 + 2)
                st, sp = g.flags()
                nc.tensor.matmul(
                    pm_out[:], ones_row[0:1, :],
                    brow_sb[0:1, C_BOUT * 128 : (C_BOUT + 2) * 128],
                    start=st, stop=sp,
                )
                for c in range(2):
                    st, sp = g.flags()
                    nc.tensor.matmul(
                        pm_out[:], h1cp[:, c, :],
                        w_all[:, O_OUT + c * 256 : O_OUT + (c + 1) * 256],
                        start=st, stop=sp,
                    )
                return pm_out

            # virtual-time pins: generous over-estimates so the scheduler
            # slots the (off-critical-path) output stores AFTER the chain
            # ops of their step, never between them
            PH1_EST_US, STEP_EST_US = 60.0, 6.8

            def wout_store(ti, pm_out):
                ms = (PH1_EST_US + (ti + 1.8) * STEP_EST_US) / 1000.0
                with tc.tile_wait_until(ms):
                    o_ = ostage.tile([128, 256], F32, tag="o")
                    nc.vector.tensor_scalar_add(o_[:], pm_out[:], 0.0)
                    nc.sync.dma_start(out[:, ti, :], o_[:])

            # ---------------- decode loop ----------------
            # PSUM tiles: r0,z0,r1,z1 own banks; hn0/hn1 share one bank
            # (tag hnx), in0/in1 share one (tag inx) — their lifetimes
            # alternate within a step.
            prev = None  # (pm_r1, pm_z1, pm_hn1, pm_in1) of step t-1
            for t in range(nsteps):
                # -- P(t): h0'(t-1)-dependent MMs (run during chain1(t-1)) --
                pm_r0 = psum.tile([128, 256], F32, tag="r0", bufs=1)
                pm_z0 = psum.tile([128, 256], F32, tag="z0", bufs=1)
                g_r0 = _Group(2 + 4 + (4 if t else 0))
                g_z0 = _Group(2 + 4 + (4 if t else 0))
                bias_mms(pm_r0, g_r0, C_RZ0 + 4 * t, 2)
                gate_mms(pm_r0, g_r0, O_RZ0, 512, (2, 3), (0, 1), (0, 1))
                bias_mms(pm_z0, g_z0, C_RZ0 + 4 * t + 2, 2)
                gate_mms(pm_z0, g_z0, O_RZ0, 512, (2, 3), (0, 1), (2, 3))

                # hn0(t) is emitted via chain1(t-1)'s after_t hook: it
                # reuses the hnx bank whose previous value (hn1(t-1))
                # chain1's t_ op reads.  The bf16 copy rides chain1's
                # mid-slot (DVE dead time under tanh).
                pm_hn0 = psum.tile([128, 256], F32, tag="hnx", bufs=1)
                g_hn0 = _Group(2 + 4)
                hn0_bf = gates.tile([128, 256], BF, tag="hnb0")

                def emit_hn0():
                    bias_mms(pm_hn0, g_hn0, C_HN0, 2)
                    gate_mms(pm_hn0, g_hn0, O_HN0, 256, (0, 1), (0, 1), (0, 1))

                def copy_hn0():
                    nc.vector.tensor_scalar_add(hn0_bf[:], pm_hn0[:], 0.0)

                # -- chain1(t-1) --
                if t:
                    chain(1, prev[0], prev[1], prev[2], prev[3],
                          after_t=emit_hn0, mid=copy_hn0, dup=h1cp_flat)
                else:
                    emit_hn0()
                    copy_hn0()

                # -- F(t): h1'(t-1)-dependent MMs --
                pm_in0 = psum.tile([128, 256], F32, tag="inx", bufs=1)
                g_in0 = _Group(2 + (4 if t else 0))
                if t:
                    # fold parts close the r0/z0 groups -> release sigmoids
                    gate_mms(pm_r0, g_r0, O_RZ0, 512, (0, 1), (2, 3), (0, 1))
                    gate_mms(pm_z0, g_z0, O_RZ0, 512, (0, 1), (2, 3), (2, 3))
                bias_mms(pm_in0, g_in0, C_IN0 + 2 * t, 2)
                pm_po = None
                if t:
                    gate_mms(pm_in0, g_in0, O_IN0, 256, (0, 1), (2, 3), (0, 1))
                    pm_po = wout_mms()
                pm_r1 = psum.tile([128, 256], F32, tag="r1", bufs=1)
                pm_z1 = psum.tile([128, 256], F32, tag="z1", bufs=1)
                g_r1 = _Group(2 + 4 + 4)
                g_z1 = _Group(2 + 4 + 4)
                bias_mms(pm_r1, g_r1, C_RZ1, 2)
                gate_mms(pm_r1, g_r1, O_RZ1, 512, (2, 3), (2, 3), (0, 1))
                bias_mms(pm_z1, g_z1, C_RZ1 + 2, 2)
                gate_mms(pm_z1, g_z1, O_RZ1, 512, (2, 3), (2, 3), (2, 3))

                # hn1(t) via chain0(t)'s after_t hook (hnx-bank rotation)
                pm_hn1 = psum.tile([128, 256], F32, tag="hnx", bufs=1)
                g_hn1 = _Group(2 + 4)
                hn1_bf = gates.tile([128, 256], BF, tag="hnb1")

                def emit_hn1():
                    bias_mms(pm_hn1, g_hn1, C_HN1, 2)
                    gate_mms(pm_hn1, g_hn1, O_HN1, 256, (0, 1), (2, 3), (0, 1))

                def copy_hn1():
                    nc.vector.tensor_scalar_add(hn1_bf[:], pm_hn1[:], 0.0)

                # -- chain0(t) --
                chain(0, pm_r0, pm_z0, hn0_bf, pm_in0, after_t=emit_hn1,
                      mid=copy_hn1)
                if pm_po is not None:
                    wout_store(t - 1, pm_po)

                # -- I(t): h0'(t)-dependent MMs --
                pm_in1 = psum.tile([128, 256], F32, tag="inx", bufs=1)
                g_in1 = _Group(2 + 4)
                gate_mms(pm_r1, g_r1, O_RZ1, 512, (0, 1), (0, 1), (0, 1))
                gate_mms(pm_z1, g_z1, O_RZ1, 512, (0, 1), (0, 1), (2, 3))
                bias_mms(pm_in1, g_in1, C_IN1, 2)
                gate_mms(pm_in1, g_in1, O_IN1, 256, (0, 1), (0, 1), (0, 1))
                prev = (pm_r1, pm_z1, hn1_bf, pm_in1)

            # -- tail: chain1(last) + its output --
            chain(1, prev[0], prev[1], prev[2], prev[3], dup=h1cp_flat)
            pm_po = wout_mms()
            wout_store(nsteps - 1, pm_po)

    _split_waits(nc)
    return nc


def _prep_inputs(encoded_features, step_emb, W_ih0, W_hh0, b_ih0, b_hh0,
                 W_ih1, W_hh1, b_ih1, b_hh1, W_out, b_out):
    """Host-side staging: slice/shard the big input, transpose + cast
    weights, fold the output projection into layer-0 input weights, fold
    the step-embedding matmul + all additive constants into bias rows."""
    f4 = np.float32
    enc_last = np.asarray(encoded_features)[:, -1].astype(BF16)
    enc_last = np.ascontiguousarray(enc_last)

    W_ih0 = np.asarray(W_ih0, f4)
    W_hh0 = np.asarray(W_hh0, f4)
    W_ih1 = np.asarray(W_ih1, f4)
    W_hh1 = np.asarray(W_hh1, f4)
    W_out = np.asarray(W_out, f4)
    step_emb = np.asarray(step_emb, f4)
    b_ih0 = np.asarray(b_ih0, f4)
    b_hh0 = np.asarray(b_hh0, f4)
    b_ih1 = np.asarray(b_ih1, f4)
    b_hh1 = np.asarray(b_hh1, f4)
    b_out = np.asarray(b_out, f4)

    W_emb = W_ih0[:, :D]          # (768, 256)
    W_pred = W_ih0[:, D:]         # (768, 256)
    W_fold = W_pred @ W_out       # (768, 256)
    b_fold = W_pred @ b_out       # (768,)

    gi_emb = step_emb[:STEPS] @ W_emb.T + b_ih0[None, :]   # (12, 768)

    def kstack(*mats_cols):
        chunks = []
        for mat, cols in mats_cols:
            mt = np.ascontiguousarray(mat.T[:, cols])  # (K, M)
            for k in range(0, mt.shape[0], 128):
                chunks.append(mt[k : k + 128])
        return np.stack(chunks).astype(BF16)  # (nk, 128, M)

    rz = slice(0, 512)
    ng = slice(512, 768)
    wrz0 = kstack((W_fold, rz), (W_hh0, rz))   # K: h1c0,h1c1,h0c0,h0c1
    win0 = kstack((W_fold, ng))
    whn0 = kstack((W_hh0, ng))
    wrz1 = kstack((W_ih1, rz), (W_hh1, rz))    # K: h0c0,h0c1,h1c0,h1c1
    win1 = kstack((W_ih1, ng))
    whn1 = kstack((W_hh1, ng))
    wout = np.stack([np.ascontiguousarray(W_out.T)[k : k + 128]
                     for k in (0, 128)]).astype(BF16)      # (2, 128, 256)

    w_pack = np.concatenate(
        [w.transpose(1, 0, 2).reshape(128, -1)
         for w in (wrz0, win0, whn0, wrz1, win1, whn1, wout)],
        axis=1,
    )
    assert w_pack.shape == (128, WCOLS)

    brows = np.zeros(NBR * 128, f4)

    def put(base, vec):
        brows[base * 128 : base * 128 + len(vec)] = vec

    for t in range(STEPS):
        extra = b_fold if t > 0 else np.zeros_like(b_fold)
        put(C_RZ0 + t * 4, gi_emb[t, :512] + b_hh0[:512] + extra[:512])
        put(C_IN0 + t * 2, gi_emb[t, 512:] + extra[512:])
    put(C_HN0, b_hh0[512:])
    put(C_RZ1, b_ih1[:512] + b_hh1[:512])
    put(C_IN1, b_ih1[512:])
    put(C_HN1, b_hh1[512:])
    put(C_BOUT, b_out)
    brows = brows.astype(BF16)[None, :]

    shared = dict(wall=np.ascontiguousarray(w_pack), brows=brows)
    in_maps = []
    for i in range(N_CORES):
        m = dict(shared)
        m["enc"] = enc_last[i * PC : (i + 1) * PC]
        in_maps.append(m)
    return in_maps


_CACHE = {}


def _run(in_maps, trace=False):
    from concourse.bass_utils import run_bass_kernel_spmd

    if "nc" not in _CACHE:
        _CACHE["nc"] = build_kernel()
    nc = _CACHE["nc"]
    res = run_bass_kernel_spmd(
        nc, in_maps, core_ids=list(range(N_CORES)), trace=trace
    )
    preds = np.concatenate([res.results[i]["out"] for i in range(N_CORES)], axis=0)
    return preds, res


def kernel(encoded_features, step_emb, W_ih0, W_hh0, b_ih0, b_hh0,
           W_ih1, W_hh1, b_ih1, b_hh1, W_out, b_out, num_steps):
    assert int(num_steps) == STEPS
    in_maps = _prep_inputs(encoded_features, step_emb, W_ih0, W_hh0, b_ih0,
                           b_hh0, W_ih1, W_hh1, b_ih1, b_hh1, W_out, b_out)
    preds, _ = _run(in_maps, trace=False)
    return preds
